# revision 47
# baseline (speedup 1.0000x reference)
"""2-layer 8-head GAT forward, distributed over 8 Trainium2 NeuronCores.

Strategy (graph data parallelism, per sharding hint):
  - Edges sorted by destination; dst nodes blocked by 128; 80 blocks sharded
    10-per-core. All index preprocessing is host-side (integers only).
  - Per layer each core holds ONE DRAM fat table (bf16):
      row n = [ h(n) (256) | alpha_src(n) (8) | alpha_dst(n) (8) | pad ] (768 B)
    built as h = x @ W_ext with attention vectors folded in
    (W_ext = [W | W@As | W@Ad | -W@Ad]).
  - Edge phase per dst block (software-pipelined prep/front/back stages):
      * batched dma_gather of fat rows by src id (4 SWDGE queues)
      * alpha_dst rows for the block's own 128 dsts: one 256B-elem gather
      * edges are sorted by dst, so per-dst edges form contiguous runs
        [start[d], start[d+1]); the dst-alpha expansion telescopes:
          sga[d,e] = (e >= start[d])      one 2x DVE tensor_scalar per block
          dx = sga @ D,  D[d] = adb[d] - adb[d-1]  (D via one tiny PE matmul
          with a constant I-minus-subdiagonal matrix)
      * selT[e,d,t] = (loc[e,t] == d) generated on-chip (2x DVE is_equal)
      * ts = s + dx; ex = exp(prelu(ts, 0.2)) via two chained ACT ops;
        rhs = [ex*h | ex] (bf16); PSUM accumulation via selT matmuls (N=264).
      * epilogue divides by summed ex, adds bias (+ ELU after layer 1).
  - Layer-2 exchange ships TRANSPOSED z (bf16, PE-transposed on the fly):
    six pipelined AllGathers of 2-2-2-2-1-1 blocks start as soon as each
    slice of z is produced and overlap the layer-1 edge phase; every core
    then rebuilds the full layer-2 fat table locally (two matmuls per
    128-node block, no further transposes), in global node order.
Output: each core writes its 1280 dst rows f32; host concatenates and trims.
"""

import os
import sys

for _p in ("/opt/trn_rl_repo", "/root/.axon_site/_ro/trn_rl_repo"):
    if os.path.isdir(_p) and _p not in sys.path:
        sys.path.append(_p)

import numpy as np

from concourse import bacc, mybir
import concourse.tile as tile
from concourse.masks import make_identity
from concourse.bass_utils import run_bass_kernel_spmd

F32 = mybir.dt.float32
BF16 = mybir.dt.bfloat16
I16 = mybir.dt.int16
AF = mybir.ActivationFunctionType
OP = mybir.AluOpType
P = 128
AG_BOUNDS = [0, 5, 10]  # block boundaries of the pipelined AllGathers


class Cfg:
    def __init__(self, n_nodes=10000, n_edges=320000, hid=256, heads=8, n_cores=8):
        self.N = n_nodes
        self.E = n_edges
        self.HID = hid
        self.H = heads
        self.C = hid // heads
        self.NC = n_cores
        self.NP = -(-n_nodes // (P * n_cores)) * (P * n_cores)
        self.NT = self.NP // P
        self.BPC = self.NT // n_cores
        self.NW = hid + 3 * heads        # table matmul width (h | s | d | -d)
        self.NW2 = hid + heads           # edge matmul rhs width (ex*h | ex)
        self.FAT = hid + 128             # fat row elems (bf16)
        assert AG_BOUNDS[-1] == self.BPC
        assert self.FAT * 2 % 256 == 0


# --------------------------------------------------------------------------
# Host preprocessing (indices / weight folding)
# --------------------------------------------------------------------------
def _wrap16(idx):
    w = idx.reshape(-1, 16).T.astype(np.int16)
    return np.tile(w, (8, 1))


def _a_expand(a, cfg):
    A = np.zeros((cfg.H, cfg.C, cfg.H), np.float32)
    for h in range(cfg.H):
        A[h, :, h] = a[h]
    return A.reshape(cfg.HID, cfg.H)


def _remap2(n, cfg):
    """Row index of node n in the slice-wise AllGather'd layer-2 table."""
    r, l = np.divmod(n, cfg.BPC * P)
    bd = np.array([x * P for x in AG_BOUNDS])
    part = np.searchsorted(bd, l, side="right") - 1
    lo, hi = bd[part], bd[np.minimum(part + 1, len(bd) - 1)]
    return cfg.NC * lo + r * (hi - lo) + (l - lo)


def preprocess(cfg, x, edges_idx, W1, a_src1, a_dst1, b1, W2, a_src2, a_dst2, b2):
    import ml_dtypes

    bfd = ml_dtypes.bfloat16

    src = np.asarray(edges_idx[0], np.int64)
    dst = np.asarray(edges_idx[1], np.int64)
    order = np.argsort(dst, kind="stable")
    src_s, dst_s = src[order], dst[order]
    blk = dst_s // P
    counts = np.bincount(blk, minlength=cfg.NT)
    T = max(1, int(-(-counts.max() // P)))
    starts = np.concatenate([[0], np.cumsum(counts)])
    EPB = T * P

    isrc = np.zeros((cfg.NC, cfg.BPC, P, 8 * T), np.int16)
    isrc2 = np.zeros((cfg.NC, cfg.BPC, P, 8 * T), np.int16)
    iblk = np.zeros((cfg.NC, cfg.BPC, P, 8), np.int16)
    iblk2 = np.zeros((cfg.NC, cfg.BPC, P, 8), np.int16)
    loce = np.zeros((cfg.NC, cfg.BPC, P, T), np.int16)
    sede = np.zeros((cfg.NC, cfg.BPC, P, 1), np.float32)
    for gb in range(cfg.NT):
        c, b = gb // cfg.BPC, gb % cfg.BPC
        s0, s1 = starts[gb], starts[gb + 1]
        n = s1 - s0
        a_src = np.zeros(EPB, np.int64)
        a_loc = np.full(EPB, -1, np.int64)
        a_src[:n] = src_s[s0:s1]
        a_loc[:n] = dst_s[s0:s1] - gb * P
        isrc[c, b] = _wrap16(a_src)
        isrc2[c, b] = _wrap16(_remap2(a_src, cfg))
        own = np.arange(gb * P, (gb + 1) * P, dtype=np.int64)
        iblk[c, b] = _wrap16(own)
        iblk2[c, b] = _wrap16(_remap2(own, cfg))
        loce[c, b] = a_loc.reshape(T, P).T            # [e, t]
        # per-dst contiguous run starts within the block's sorted edges
        cnt_d = np.bincount(a_loc[:n], minlength=P)
        sede[c, b, :, 0] = (np.cumsum(cnt_d) - cnt_d).astype(np.float32)

    Wd1 = W1 @ _a_expand(a_dst1, cfg)
    W1e = np.concatenate(
        [W1, W1 @ _a_expand(a_src1, cfg), Wd1, -Wd1], axis=1
    ).astype(np.float32)
    Wd2 = W2 @ _a_expand(a_dst2, cfg)
    W2e = np.concatenate(
        [W2, W2 @ _a_expand(a_src2, cfg), Wd2, -Wd2], axis=1
    ).astype(np.float32)

    xT = np.zeros((cfg.HID, cfg.NP), np.float32)
    xT[:, : cfg.N] = np.asarray(x, np.float32).T
    b1b = np.broadcast_to(np.asarray(b1, np.float32), (P, cfg.HID)).copy()
    b2b = np.broadcast_to(np.asarray(b2, np.float32), (P, cfg.HID)).copy()

    shared = {
        "xT": xT.astype(bfd), "w1e": W1e.astype(bfd), "w2e": W2e.astype(bfd),
        "b1b": b1b, "b2b": b2b,
    }
    zero_bias = bool(np.all(b1b == 0) and np.all(b2b == 0))
    cpb = counts.reshape(cfg.NC, cfg.BPC)
    tbp = [int(-(-cpb[:, b].max() // P)) for b in range(cfg.BPC)]
    in_maps = [
        dict(shared, isrc=isrc[c], isrc2=isrc2[c], iblk=iblk[c],
             iblk2=iblk2[c], loce=loce[c], sede=sede[c])
        for c in range(cfg.NC)
    ]
    return in_maps, T, zero_bias, tbp


# --------------------------------------------------------------------------
# Device program
# --------------------------------------------------------------------------
def _edge_phase(nc, tc, cfg, T, cn, layer, tabf, isrc_e, iblk_e, loc_e, se_e,
                fat2own=None, tabf2sh=None, out_e=None):
    """Edge phase for one layer, software-pipelined over this core's blocks.

    layer==1: epilogue applies ELU, PE-transposes z, stages zT slices for
    the pipelined AllGathers, then (after the block loop) rebuilds the full
    layer-2 fat table from the gathered zT slices.
    layer==2: epilogue writes the final f32 output rows.
    """
    HID, H, C, NW2, FAT, BPC = cfg.HID, cfg.H, cfg.C, cfg.NW2, cfg.FAT, cfg.BPC
    CH = 8  # 1024-idx gather chunks (HW limit)
    qn = [0]
    bias_t = cn["b1t"] if layer == 1 else cn["b2t"]
    st = {}  # per-block live tiles

    with (
        tc.tile_pool(name=f"ge{layer}", bufs=3) as gp,
        tc.tile_pool(name=f"rh{layer}", bufs=3) as rp,
        tc.tile_pool(name=f"ix{layer}", bufs=3) as ip,
        tc.tile_pool(name=f"sl{layer}", bufs=2) as slp,
        tc.tile_pool(name=f"wk{layer}", bufs=3) as wp,
        tc.tile_pool(name=f"eo{layer}", bufs=3) as op_,
        tc.tile_pool(name=f"eps{layer}", bufs=3, space="PSUM") as pp,
        tc.tile_pool(name=f"dps{layer}", bufs=3, space="PSUM") as dpp,
        tc.tile_pool(name=f"l2ps{layer}", bufs=1, space="PSUM") as l2pp,
        tc.tile_pool(name=f"l2sb{layer}", bufs=3) as l2sp,
    ):
        def loads(b):
            s = st[b] = {}
            ist = ip.tile([P, 8 * T], I16, tag="isrc")
            nc.sync.dma_start(ist[:], isrc_e[b])
            loc = ip.tile([P, T], I16, tag="loc")
            nc.sync.dma_start(loc[:], loc_e[b])
            se = ip.tile([P, 1], F32, tag="se")
            nc.sync.dma_start(se[:], se_e[b])
            ibt = ip.tile([P, 8], I16, tag="iblk")
            nc.sync.dma_start(ibt[:], iblk_e[b])
            s["ist"], s["loc"], s["se"], s["ibt"] = ist, loc, se, ibt

        def prep(b):
            s = st[b]
            Tb = cn["tbp"][b]
            ist, loc, se, ibt = s["ist"], s["loc"], s["se"], s["ibt"]
            # on-chip selection masks (all operands packed 16-bit last dim)
            slt = slp.tile([P, P, T], BF16, tag="slt")   # [e, d, t]
            nc.vector.tensor_tensor(
                slt[:, :, 0:Tb],
                loc[:, 0:Tb].to_broadcast([P, Tb, P]).rearrange(
                    "p t d -> p d t"),
                cn["iota_dt"][:, :, 0:Tb], op=OP.is_equal,
            )
            sga = slp.tile([P, T * P], BF16, tag="sga")  # [d, e] >= start
            nc.vector.tensor_scalar(
                sga[:, 0 : Tb * P], cn["iota_e"][:, 0 : Tb * P], se[:, 0:1],
                None, op0=OP.is_ge,
            )
            s["slt"], s["sga"] = slt, sga
            # alpha_dst for the block's 128 dsts: 256B-elem gather of the
            # tail half of the fat rows ([s | d | -d | pad])
            adb = ip.tile([P, 1, P], BF16, tag="adb")
            nc.gpsimd.dma_gather(
                out_ap=adb[:], in_ap=tabf[:, HID:FAT], idxs_ap=ibt[:],
                num_idxs=P, num_idxs_reg=P, elem_size=P, elem_step=FAT,
                queue_num=qn[0] % 4,
            )
            qn[0] += 1
            s["adb"] = adb
            # fat-row gather by src id
            gA = gp.tile([P, T, FAT], BF16, tag="gA")
            for c0 in range(0, Tb, CH):
                cw = min(CH, Tb - c0)
                nc.gpsimd.dma_gather(
                    out_ap=gA[:, c0 : c0 + cw, :], in_ap=tabf[:, :],
                    idxs_ap=ist[:, c0 * 8 : (c0 + cw) * 8],
                    num_idxs=P * cw, num_idxs_reg=P * cw, elem_size=FAT,
                    queue_num=qn[0] % 4,
                )
                qn[0] += 1
            s["gA"] = gA

        def front(b):
            s = st[b]
            Tb = cn["tbp"][b]
            gA, slt, sga, adb = s["gA"], s["slt"], s["sga"], s["adb"]
            # D[d] = adb[d] - adb[d-1] via the constant shift-diff matrix,
            # packed into the tail columns of the dx PSUM tile
            dx = dpp.tile([P, (T + 1) * H], F32, tag="dx")
            dps = dx[:, T * H : (T + 1) * H]
            nc.tensor.matmul(dps, cn["sdif"][:], adb[:, 0, H : 2 * H],
                             start=True, stop=True)
            dal = l2sp.tile([P, H], BF16, tag="dal")
            nc.scalar.activation(dal[:], dps, AF.Copy)
            # telescoped dst-alpha expansion: dx = sga @ D
            for t in range(Tb):
                nc.tensor.matmul(
                    dx[:, t * H : (t + 1) * H], sga[:, t * P : (t + 1) * P],
                    dal[:], start=True, stop=True,
                )
            ts = wp.tile([P, T, H], F32, tag="ts")
            nc.vector.tensor_tensor(
                ts[:, 0:Tb, :], gA[:, 0:Tb, HID : HID + H],
                dx[:, 0 : Tb * H].rearrange("p (t h) -> p t h", t=Tb),
                op=OP.add,
            )
            # ex = exp(leaky_relu(ts)): two chained ACT ops
            lr = wp.tile([P, T, H], F32, tag="lr")
            nc.scalar.activation(lr[:, 0:Tb, :], ts[:, 0:Tb, :], AF.Prelu,
                                 alpha=0.2)
            rhs = rp.tile([P, T, NW2], BF16, tag="rhs")
            exs = rhs[:, 0:Tb, HID : HID + H]
            nc.scalar.activation(exs, lr[:, 0:Tb, :], AF.Exp)
            nc.vector.tensor_tensor(
                rhs[:, 0:Tb, 0:HID].rearrange("p t (h c) -> p t h c", h=H),
                gA[:, 0:Tb, 0:HID].rearrange("p t (h c) -> p t h c", h=H),
                exs.rearrange("p t (h o) -> p t h o", h=H).to_broadcast(
                    [P, Tb, H, C]
                ),
                op=OP.mult,
            )
            s["rhs"] = rhs

        def back(b):
            s = st.pop(b)
            Tb = cn["tbp"][b]
            slt, rhs = s["slt"], s["rhs"]
            ps = pp.tile([P, NW2], F32, tag="eps")
            for t in range(Tb):
                nc.tensor.matmul(
                    ps[:], slt[:, :, t], rhs[:, t, :],
                    start=(t == 0), stop=(t == Tb - 1),
                )
            den = op_.tile([P, H], F32, tag="den")
            nc.vector.tensor_scalar_add(den[:], ps[:, HID : HID + H], 1e-16)
            rec = op_.tile([P, H], F32, tag="rec")
            nc.vector.reciprocal(rec[:], den[:])
            ot = op_.tile([P, HID], F32, tag="ot")
            nc.vector.tensor_tensor(
                ot[:].rearrange("p (h c) -> p h c", h=H),
                ps[:, 0:HID].rearrange("p (h c) -> p h c", h=H),
                rec[:].to_broadcast([P, H, C]),
                op=OP.mult,
            )
            if not cn["zero_bias"]:
                nc.vector.tensor_tensor(ot[:], ot[:], bias_t[:], op=OP.add)
            if layer == 1:
                # ELU(x) = relu(x) + exp(min(x,0)) - 1 -> bf16 z
                r_ = op_.tile([P, HID], F32, tag="relu")
                nc.scalar.activation(r_[:], ot[:], AF.Relu)
                m_ = op_.tile([P, HID], F32, tag="mneg")
                nc.vector.tensor_scalar(m_[:], ot[:], 0.0, None, op0=OP.min)
                nc.scalar.activation(m_[:], m_[:], AF.Exp)
                nc.vector.tensor_scalar_add(m_[:], m_[:], -1.0)
                zt = op_.tile([P, HID], BF16, tag="zt")
                nc.vector.tensor_tensor(zt[:], r_[:], m_[:], op=OP.add)
                # build this block's layer-2 table rows locally:
                # transpose z on the PE, then fold-in W2_ext
                psT = l2pp.tile([P, 2 * P], BF16, tag="psT")
                nc.tensor.transpose(psT[:, 0:P], zt[:, 0:P], cn["idn"][:])
                nc.tensor.transpose(psT[:, P : 2 * P], zt[:, P : 2 * P],
                                    cn["idn"][:])
                zT = l2sp.tile([P, 2 * P], BF16, tag="zT")
                nc.scalar.activation(zT[:], psT[:], AF.Copy)
                ps2 = l2pp.tile([P, cfg.NW], F32, tag="ps2")
                nc.tensor.matmul(ps2[:], zT[:, 0:P], cn["w2a"][:],
                                 start=True, stop=False)
                nc.tensor.matmul(ps2[:], zT[:, P : 2 * P], cn["w2b"][:],
                                 start=False, stop=True)
                f2 = l2sp.tile([P, FAT], BF16, tag="f2")
                nc.scalar.activation(f2[:, 0 : cfg.NW], ps2[:], AF.Copy)
                nc.scalar.dma_start(fat2own[b * P : (b + 1) * P, :], f2[:])
            else:
                nc.sync.dma_start(out_e[b * P : (b + 1) * P, :], ot[:])

        def maybe_ag(b):
            if layer != 1:
                return
            bd = [x * P for x in AG_BOUNDS]
            for k in range(len(bd) - 1):
                if b + 1 == bd[k + 1] // P:
                    nc.gpsimd.collective_compute(
                        "AllGather", OP.bypass,
                        replica_groups=[list(range(cfg.NC))],
                        ins=[fat2own[bd[k] : bd[k + 1], :]],
                        outs=[tabf2sh[cfg.NC * bd[k] : cfg.NC * bd[k + 1], :]],
                    )

        loads(0)
        loads(1)
        prep(0)
        for i in range(BPC):
            if i + 2 < BPC:
                loads(i + 2)
            if i + 1 < BPC:
                prep(i + 1)
            front(i)
            if i >= 1:
                back(i - 1)
                maybe_ag(i - 1)
        back(BPC - 1)
        maybe_ag(BPC - 1)



def build_program(cfg, T, zero_bias=False, tbp=None):
    nc = bacc.Bacc(num_swdge_queues=4)
    HID, NW, NP, NT, BPC = cfg.HID, cfg.NW, cfg.NP, cfg.NT, cfg.BPC

    xT_e = nc.declare_dram_parameter("xT", [HID, NP], BF16, isOutput=False)
    w1_e = nc.declare_dram_parameter("w1e", [HID, NW], BF16, isOutput=False)
    w2_e = nc.declare_dram_parameter("w2e", [HID, NW], BF16, isOutput=False)
    b1_e = nc.declare_dram_parameter("b1b", [P, HID], F32, isOutput=False)
    b2_e = nc.declare_dram_parameter("b2b", [P, HID], F32, isOutput=False)
    isrc_e = nc.declare_dram_parameter("isrc", [BPC, P, 8 * T], I16, isOutput=False)
    isrc2_e = nc.declare_dram_parameter("isrc2", [BPC, P, 8 * T], I16, isOutput=False)
    iblk_e = nc.declare_dram_parameter("iblk", [BPC, P, 8], I16, isOutput=False)
    iblk2_e = nc.declare_dram_parameter("iblk2", [BPC, P, 8], I16, isOutput=False)
    loc_e = nc.declare_dram_parameter("loce", [BPC, P, T], I16, isOutput=False)
    se_e = nc.declare_dram_parameter("sede", [BPC, P, 1], F32, isOutput=False)
    out_e = nc.declare_dram_parameter("out", [BPC * P, HID], F32, isOutput=True)

    tabf1 = nc.dram_tensor("tabf1", [NP, cfg.FAT], BF16)
    fat2own = nc.dram_tensor("fat2own", [BPC * P, cfg.FAT], BF16)
    tabf2sh = nc.dram_tensor("tabf2sh", [NP, cfg.FAT], BF16,
                             addr_space="Shared")

    with tile.TileContext(nc) as tc:
        with tc.tile_pool(name="const", bufs=1) as cp:
            cn = {}
            for nm, src in (("w1a", w1_e), ("w1b", w1_e), ("w2a", w2_e),
                            ("w2b", w2_e)):
                t = cp.tile([P, NW], BF16, tag=nm)
                lo = 0 if nm.endswith("a") else P
                nc.sync.dma_start(t[:], src[lo : lo + P, :])
                cn[nm] = t
            b1t = cp.tile([P, HID], F32)
            nc.sync.dma_start(b1t[:], b1_e[:, :])
            cn["b1t"] = b1t
            b2t = cp.tile([P, HID], F32)
            nc.sync.dma_start(b2t[:], b2_e[:, :])
            cn["b2t"] = b2t
            idn = cp.tile([P, P], BF16)
            make_identity(nc, idn[:])
            cn["idn"] = idn
            # I minus subdiagonal: D = sdif.T @ v telescopes run-starts
            sdif = cp.tile([P, P], BF16)
            make_identity(nc, sdif[:])
            nc.gpsimd.affine_select(
                out=sdif[:], in_=sdif[:], compare_op=OP.not_equal,
                fill=-1.0, base=1, pattern=[[-1, P]], channel_multiplier=1,
            )
            cn["sdif"] = sdif
            iota_dt = cp.tile([P, P, T], I16)
            nc.gpsimd.iota(iota_dt[:], pattern=[[1, P], [0, T]],
                           channel_multiplier=0)
            cn["iota_dt"] = iota_dt
            iota_e = cp.tile([P, T * P], I16)
            nc.gpsimd.iota(iota_e[:], pattern=[[1, T * P]],
                           channel_multiplier=0)
            cn["iota_e"] = iota_e

            # ---- layer-1 tables (full, redundant per core) ----
            PAN = 10
            with (
                tc.tile_pool(name="s1", bufs=3) as sp,
                tc.tile_pool(name="fp1", bufs=6) as fp,
                tc.tile_pool(name="ps1", bufs=6, space="PSUM") as pp,
            ):
                for pan in range(-(-NT // PAN)):
                    j0, j1 = pan * PAN, min(NT, (pan + 1) * PAN)
                    w = (j1 - j0) * P
                    xp0 = sp.tile([P, PAN * P], BF16, tag="xp0")
                    nc.sync.dma_start(xp0[:, :w], xT_e[0:P, j0 * P : j1 * P])
                    xp1 = sp.tile([P, PAN * P], BF16, tag="xp1")
                    nc.sync.dma_start(xp1[:, :w], xT_e[P : 2 * P, j0 * P : j1 * P])
                    for j in range(j0, j1):
                        o = (j - j0) * P
                        ps = pp.tile([P, NW], F32, tag="tps")
                        nc.tensor.matmul(ps[:], xp0[:, o : o + P], cn["w1a"][:],
                                         start=True, stop=False)
                        nc.tensor.matmul(ps[:], xp1[:, o : o + P], cn["w1b"][:],
                                         start=False, stop=True)
                        fat = fp.tile([P, cfg.FAT], BF16, tag="fat")
                        if j % 2 == 0:
                            nc.vector.tensor_copy(fat[:, 0:NW], ps[:])
                            nc.sync.dma_start(
                                tabf1[j * P : (j + 1) * P, :], fat[:])
                        else:
                            nc.scalar.activation(fat[:, 0:NW], ps[:], AF.Copy)
                            nc.scalar.dma_start(
                                tabf1[j * P : (j + 1) * P, :], fat[:])

            cn["zero_bias"] = zero_bias
            cn["tbp"] = tbp or [T] * cfg.BPC
            _edge_phase(nc, tc, cfg, T, cn, 1, tabf1, isrc_e, iblk_e, loc_e,
                        se_e, fat2own=fat2own, tabf2sh=tabf2sh)
            _edge_phase(nc, tc, cfg, T, cn, 2, tabf2sh, isrc2_e, iblk2_e,
                        loc_e, se_e, out_e=out_e)
    nc.finalize()
    return nc


# --------------------------------------------------------------------------
# Entry point
# --------------------------------------------------------------------------
def run_gat(inputs, cfg=None, trace=False):
    cfg = cfg or Cfg()
    in_maps, T, zero_bias, tbp = preprocess(cfg, **inputs)
    nc = build_program(cfg, T, zero_bias, tbp)
    res = run_bass_kernel_spmd(nc, in_maps, list(range(cfg.NC)), trace=trace)
    out = np.concatenate([res.results[c]["out"] for c in range(cfg.NC)], axis=0)
    return out[: cfg.N], res


def kernel(**inputs) -> np.ndarray:
    out, _ = run_gat(inputs)
    return np.ascontiguousarray(out, dtype=np.float32)


# revision 48
# speedup vs baseline: 1.0259x; 1.0259x over previous
"""2-layer 8-head GAT forward, distributed over 8 Trainium2 NeuronCores.

Strategy (graph data parallelism, per sharding hint):
  - Edges sorted by destination; dst nodes blocked by 128; 80 blocks sharded
    10-per-core. All index preprocessing is host-side (integers only).
  - Per layer each core holds ONE DRAM fat table (bf16):
      row n = [ h(n) (256) | alpha_src(n) (8) | alpha_dst(n) (8) | pad ] (768 B)
    built as h = x @ W_ext with attention vectors folded in
    (W_ext = [W | W@As | W@Ad | -W@Ad]).
  - Edge phase per dst block (software-pipelined prep/front/back stages):
      * batched dma_gather of fat rows by src id (4 SWDGE queues)
      * alpha_dst rows for the block's own 128 dsts: one 256B-elem gather
      * edges are sorted by dst, so per-dst edges form contiguous runs
        [start[d], start[d+1]); the dst-alpha expansion telescopes:
          sga[d,e] = (e >= start[d])      one 2x DVE tensor_scalar per block
          dx = sga @ D,  D[d] = adb[d] - adb[d-1]  (D via one tiny PE matmul
          with a constant I-minus-subdiagonal matrix)
      * selT[e,d,t] = (loc[e,t] == d) generated on-chip (2x DVE is_equal)
      * ts = s + dx; ex = exp(prelu(ts, 0.2)) via two chained ACT ops;
        rhs = [ex*h | ex] (bf16); PSUM accumulation via selT matmuls (N=264).
      * epilogue divides by summed ex, adds bias (+ ELU after layer 1).
  - Layer-2 exchange ships TRANSPOSED z (bf16, PE-transposed on the fly):
    six pipelined AllGathers of 2-2-2-2-1-1 blocks start as soon as each
    slice of z is produced and overlap the layer-1 edge phase; every core
    then rebuilds the full layer-2 fat table locally (two matmuls per
    128-node block, no further transposes), in global node order.
Output: each core writes its 1280 dst rows f32; host concatenates and trims.
"""

import os
import sys

for _p in ("/opt/trn_rl_repo", "/root/.axon_site/_ro/trn_rl_repo"):
    if os.path.isdir(_p) and _p not in sys.path:
        sys.path.append(_p)

import numpy as np

from concourse import bacc, mybir
import concourse.tile as tile
from concourse.masks import make_identity
from concourse.bass_utils import run_bass_kernel_spmd

F32 = mybir.dt.float32
BF16 = mybir.dt.bfloat16
I16 = mybir.dt.int16
AF = mybir.ActivationFunctionType
OP = mybir.AluOpType
P = 128
AG_BOUNDS = [0, 5, 10]  # block boundaries of the pipelined AllGathers


class Cfg:
    def __init__(self, n_nodes=10000, n_edges=320000, hid=256, heads=8, n_cores=8):
        self.N = n_nodes
        self.E = n_edges
        self.HID = hid
        self.H = heads
        self.C = hid // heads
        self.NC = n_cores
        self.NP = -(-n_nodes // (P * n_cores)) * (P * n_cores)
        self.NT = self.NP // P
        self.BPC = self.NT // n_cores
        self.NW = hid + 3 * heads        # table matmul width (h | s | d | -d)
        self.NW2 = hid + heads           # edge matmul rhs width (ex*h | ex)
        self.FAT = hid + 128             # fat row elems (bf16)
        assert AG_BOUNDS[-1] == self.BPC
        assert self.FAT * 2 % 256 == 0


# --------------------------------------------------------------------------
# Host preprocessing (indices / weight folding)
# --------------------------------------------------------------------------
def _wrap16(idx):
    w = idx.reshape(-1, 16).T.astype(np.int16)
    return np.tile(w, (8, 1))


def _a_expand(a, cfg):
    A = np.zeros((cfg.H, cfg.C, cfg.H), np.float32)
    for h in range(cfg.H):
        A[h, :, h] = a[h]
    return A.reshape(cfg.HID, cfg.H)


def _remap2(n, cfg):
    """Row index of node n in the slice-wise AllGather'd layer-2 table."""
    r, l = np.divmod(n, cfg.BPC * P)
    bd = np.array([x * P for x in AG_BOUNDS])
    part = np.searchsorted(bd, l, side="right") - 1
    lo, hi = bd[part], bd[np.minimum(part + 1, len(bd) - 1)]
    return cfg.NC * lo + r * (hi - lo) + (l - lo)


def preprocess(cfg, x, edges_idx, W1, a_src1, a_dst1, b1, W2, a_src2, a_dst2, b2):
    import ml_dtypes

    bfd = ml_dtypes.bfloat16

    src = np.asarray(edges_idx[0], np.int64)
    dst = np.asarray(edges_idx[1], np.int64)
    order = np.argsort(dst, kind="stable")
    src_s, dst_s = src[order], dst[order]
    blk = dst_s // P
    counts = np.bincount(blk, minlength=cfg.NT)
    T = max(1, int(-(-counts.max() // P)))
    starts = np.concatenate([[0], np.cumsum(counts)])
    EPB = T * P

    isrc = np.zeros((cfg.NC, cfg.BPC, P, 8 * T), np.int16)
    isrc2 = np.zeros((cfg.NC, cfg.BPC, P, 8 * T), np.int16)
    iblk = np.zeros((cfg.NC, cfg.BPC, P, 8), np.int16)
    iblk2 = np.zeros((cfg.NC, cfg.BPC, P, 8), np.int16)
    loce = np.zeros((cfg.NC, cfg.BPC, P, T), np.int16)
    sede = np.zeros((cfg.NC, cfg.BPC, P, 1), np.float32)
    for gb in range(cfg.NT):
        c, b = gb // cfg.BPC, gb % cfg.BPC
        s0, s1 = starts[gb], starts[gb + 1]
        n = s1 - s0
        a_src = np.zeros(EPB, np.int64)
        a_loc = np.full(EPB, -1, np.int64)
        a_src[:n] = src_s[s0:s1]
        a_loc[:n] = dst_s[s0:s1] - gb * P
        isrc[c, b] = _wrap16(a_src)
        isrc2[c, b] = _wrap16(_remap2(a_src, cfg))
        own = np.arange(gb * P, (gb + 1) * P, dtype=np.int64)
        iblk[c, b] = _wrap16(own)
        iblk2[c, b] = _wrap16(_remap2(own, cfg))
        loce[c, b] = a_loc.reshape(T, P).T            # [e, t]
        # per-dst contiguous run starts within the block's sorted edges
        cnt_d = np.bincount(a_loc[:n], minlength=P)
        sede[c, b, :, 0] = (np.cumsum(cnt_d) - cnt_d).astype(np.float32)

    Wd1 = W1 @ _a_expand(a_dst1, cfg)
    W1e = np.concatenate(
        [W1, W1 @ _a_expand(a_src1, cfg), Wd1, -Wd1], axis=1
    ).astype(np.float32)
    Wd2 = W2 @ _a_expand(a_dst2, cfg)
    W2e = np.concatenate(
        [W2, W2 @ _a_expand(a_src2, cfg), Wd2, -Wd2], axis=1
    ).astype(np.float32)

    xT = np.zeros((cfg.HID, cfg.NP), np.float32)
    xT[:, : cfg.N] = np.asarray(x, np.float32).T
    b1b = np.broadcast_to(np.asarray(b1, np.float32), (P, cfg.HID)).copy()
    b2b = np.broadcast_to(np.asarray(b2, np.float32), (P, cfg.HID)).copy()

    shared = {
        "xT": xT.astype(bfd), "w1e": W1e.astype(bfd), "w2e": W2e.astype(bfd),
        "b1b": b1b, "b2b": b2b,
    }
    zero_bias = bool(np.all(b1b == 0) and np.all(b2b == 0))
    cpb = counts.reshape(cfg.NC, cfg.BPC)
    tbp = [int(-(-cpb[:, b].max() // P)) for b in range(cfg.BPC)]
    in_maps = [
        dict(shared, isrc=isrc[c], isrc2=isrc2[c], iblk=iblk[c],
             iblk2=iblk2[c], loce=loce[c], sede=sede[c])
        for c in range(cfg.NC)
    ]
    return in_maps, T, zero_bias, tbp


# --------------------------------------------------------------------------
# Device program
# --------------------------------------------------------------------------
def _edge_phase(nc, tc, cfg, T, cn, layer, tabf, isrc_e, iblk_e, loc_e, se_e,
                fat2own=None, tabf2sh=None, out_e=None):
    """Edge phase for one layer, software-pipelined over this core's blocks.

    layer==1: epilogue applies ELU, PE-transposes z, stages zT slices for
    the pipelined AllGathers, then (after the block loop) rebuilds the full
    layer-2 fat table from the gathered zT slices.
    layer==2: epilogue writes the final f32 output rows.
    """
    HID, H, C, NW2, FAT, BPC = cfg.HID, cfg.H, cfg.C, cfg.NW2, cfg.FAT, cfg.BPC
    CH = 8  # 1024-idx gather chunks (HW limit)
    qn = [0]
    bias_t = cn["b1t"] if layer == 1 else cn["b2t"]
    st = {}  # per-block live tiles

    with (
        tc.tile_pool(name=f"ge{layer}", bufs=3) as gp,
        tc.tile_pool(name=f"rh{layer}", bufs=3) as rp,
        tc.tile_pool(name=f"ix{layer}", bufs=3) as ip,
        tc.tile_pool(name=f"sl{layer}", bufs=2) as slp,
        tc.tile_pool(name=f"wk{layer}", bufs=3) as wp,
        tc.tile_pool(name=f"eo{layer}", bufs=3) as op_,
        tc.tile_pool(name=f"eps{layer}", bufs=3, space="PSUM") as pp,
        tc.tile_pool(name=f"dps{layer}", bufs=2, space="PSUM") as dpp,
        tc.tile_pool(name=f"l2ps{layer}", bufs=1, space="PSUM") as l2pp,
        tc.tile_pool(name=f"l2sb{layer}", bufs=3) as l2sp,
    ):
        def loads(b):
            s = st[b] = {}
            ist = ip.tile([P, 8 * T], I16, tag="isrc")
            nc.sync.dma_start(ist[:], isrc_e[b])
            loc = ip.tile([P, T], I16, tag="loc")
            nc.sync.dma_start(loc[:], loc_e[b])
            se = ip.tile([P, 1], F32, tag="se")
            nc.sync.dma_start(se[:], se_e[b])
            ibt = ip.tile([P, 8], I16, tag="iblk")
            nc.sync.dma_start(ibt[:], iblk_e[b])
            s["ist"], s["loc"], s["se"], s["ibt"] = ist, loc, se, ibt

        def prep(b):
            s = st[b]
            Tb = cn["tbp"][b]
            ist, loc, se, ibt = s["ist"], s["loc"], s["se"], s["ibt"]
            # on-chip selection masks (all operands packed 16-bit last dim)
            slt = slp.tile([P, P, T], BF16, tag="slt")   # [e, d, t]
            nc.vector.tensor_tensor(
                slt[:, :, 0:Tb],
                loc[:, 0:Tb].to_broadcast([P, Tb, P]).rearrange(
                    "p t d -> p d t"),
                cn["iota_dt"][:, :, 0:Tb], op=OP.is_equal,
            )
            sga = slp.tile([P, T * P], BF16, tag="sga")  # [d, e] >= start
            nc.vector.tensor_scalar(
                sga[:, 0 : Tb * P], cn["iota_e"][:, 0 : Tb * P], se[:, 0:1],
                None, op0=OP.is_ge,
            )
            s["slt"], s["sga"] = slt, sga
            # alpha_dst for the block's 128 dsts: 256B-elem gather of the
            # tail half of the fat rows ([s | d | -d | pad])
            adb = ip.tile([P, 1, P], BF16, tag="adb")
            nc.gpsimd.dma_gather(
                out_ap=adb[:], in_ap=tabf[:, HID:FAT], idxs_ap=ibt[:],
                num_idxs=P, num_idxs_reg=P, elem_size=P, elem_step=FAT,
                queue_num=qn[0] % 4,
            )
            qn[0] += 1
            s["adb"] = adb
            # fat-row gather by src id
            gA = gp.tile([P, T, FAT], BF16, tag="gA")
            for c0 in range(0, Tb, CH):
                cw = min(CH, Tb - c0)
                nc.gpsimd.dma_gather(
                    out_ap=gA[:, c0 : c0 + cw, :], in_ap=tabf[:, :],
                    idxs_ap=ist[:, c0 * 8 : (c0 + cw) * 8],
                    num_idxs=P * cw, num_idxs_reg=P * cw, elem_size=FAT,
                    queue_num=qn[0] % 4,
                )
                qn[0] += 1
            s["gA"] = gA

        def front(b):
            s = st[b]
            Tb = cn["tbp"][b]
            gA, slt, sga, adb = s["gA"], s["slt"], s["sga"], s["adb"]
            # D[d] = adb[d] - adb[d-1] via the constant shift-diff matrix
            dps = l2pp.tile([P, H], F32, tag="dps")
            nc.tensor.matmul(dps[:], cn["sdif"][:], adb[:, 0, H : 2 * H],
                             start=True, stop=True)
            dal = l2sp.tile([P, H], BF16, tag="dal")
            nc.scalar.activation(dal[:], dps[:], AF.Copy)
            # telescoped dst-alpha expansion: dx = sga @ D
            dx = dpp.tile([P, T * H], F32, tag="dx")
            for t in range(Tb):
                nc.tensor.matmul(
                    dx[:, t * H : (t + 1) * H], sga[:, t * P : (t + 1) * P],
                    dal[:], start=True, stop=True,
                )
            ts = wp.tile([P, T, H], F32, tag="ts")
            nc.vector.tensor_tensor(
                ts[:, 0:Tb, :], gA[:, 0:Tb, HID : HID + H],
                dx[:, 0 : Tb * H].rearrange("p (t h) -> p t h", t=Tb),
                op=OP.add,
            )
            # ex = exp(leaky_relu(ts)): two chained ACT ops
            lr = wp.tile([P, T, H], F32, tag="lr")
            nc.scalar.activation(lr[:, 0:Tb, :], ts[:, 0:Tb, :], AF.Prelu,
                                 alpha=0.2)
            rhs = rp.tile([P, T, NW2], BF16, tag="rhs")
            exs = rhs[:, 0:Tb, HID : HID + H]
            nc.scalar.activation(exs, lr[:, 0:Tb, :], AF.Exp)
            nc.vector.tensor_tensor(
                rhs[:, 0:Tb, 0:HID].rearrange("p t (h c) -> p t h c", h=H),
                gA[:, 0:Tb, 0:HID].rearrange("p t (h c) -> p t h c", h=H),
                exs.rearrange("p t (h o) -> p t h o", h=H).to_broadcast(
                    [P, Tb, H, C]
                ),
                op=OP.mult,
            )
            s["rhs"] = rhs

        def back(b):
            s = st.pop(b)
            Tb = cn["tbp"][b]
            slt, rhs = s["slt"], s["rhs"]
            ps = pp.tile([P, NW2], F32, tag="eps")
            for t in range(Tb):
                nc.tensor.matmul(
                    ps[:], slt[:, :, t], rhs[:, t, :],
                    start=(t == 0), stop=(t == Tb - 1),
                )
            den = op_.tile([P, H], F32, tag="den")
            nc.vector.tensor_scalar_add(den[:], ps[:, HID : HID + H], 1e-16)
            rec = op_.tile([P, H], F32, tag="rec")
            nc.vector.reciprocal(rec[:], den[:])
            ot = op_.tile([P, HID], F32, tag="ot")
            nc.vector.tensor_tensor(
                ot[:].rearrange("p (h c) -> p h c", h=H),
                ps[:, 0:HID].rearrange("p (h c) -> p h c", h=H),
                rec[:].to_broadcast([P, H, C]),
                op=OP.mult,
            )
            if not cn["zero_bias"]:
                nc.vector.tensor_tensor(ot[:], ot[:], bias_t[:], op=OP.add)
            if layer == 1:
                # ELU(x) = relu(x) + exp(min(x,0)) - 1 -> bf16 z
                r_ = op_.tile([P, HID], F32, tag="relu")
                nc.scalar.activation(r_[:], ot[:], AF.Relu)
                m_ = op_.tile([P, HID], F32, tag="mneg")
                nc.vector.tensor_scalar(m_[:], ot[:], 0.0, None, op0=OP.min)
                nc.scalar.activation(m_[:], m_[:], AF.Exp)
                nc.vector.tensor_scalar_add(m_[:], m_[:], -1.0)
                zt = op_.tile([P, HID], BF16, tag="zt")
                nc.vector.tensor_tensor(zt[:], r_[:], m_[:], op=OP.add)
                # build this block's layer-2 table rows locally:
                # transpose z on the PE, then fold-in W2_ext
                psT = l2pp.tile([P, 2 * P], BF16, tag="psT")
                nc.tensor.transpose(psT[:, 0:P], zt[:, 0:P], cn["idn"][:])
                nc.tensor.transpose(psT[:, P : 2 * P], zt[:, P : 2 * P],
                                    cn["idn"][:])
                zT = l2sp.tile([P, 2 * P], BF16, tag="zT")
                nc.scalar.activation(zT[:], psT[:], AF.Copy)
                ps2 = l2pp.tile([P, cfg.NW], F32, tag="ps2")
                nc.tensor.matmul(ps2[:], zT[:, 0:P], cn["w2a"][:],
                                 start=True, stop=False)
                nc.tensor.matmul(ps2[:], zT[:, P : 2 * P], cn["w2b"][:],
                                 start=False, stop=True)
                f2 = l2sp.tile([P, FAT], BF16, tag="f2")
                nc.scalar.activation(f2[:, 0 : cfg.NW], ps2[:], AF.Copy)
                nc.scalar.dma_start(fat2own[b * P : (b + 1) * P, :], f2[:])
            else:
                nc.sync.dma_start(out_e[b * P : (b + 1) * P, :], ot[:])

        def maybe_ag(b):
            if layer != 1:
                return
            bd = [x * P for x in AG_BOUNDS]
            for k in range(len(bd) - 1):
                if b + 1 == bd[k + 1] // P:
                    nc.gpsimd.collective_compute(
                        "AllGather", OP.bypass,
                        replica_groups=[list(range(cfg.NC))],
                        ins=[fat2own[bd[k] : bd[k + 1], :]],
                        outs=[tabf2sh[cfg.NC * bd[k] : cfg.NC * bd[k + 1], :]],
                    )

        loads(0)
        loads(1)
        prep(0)
        for i in range(BPC):
            if i + 2 < BPC:
                loads(i + 2)
            if i + 1 < BPC:
                prep(i + 1)
            front(i)
            if i >= 1:
                back(i - 1)
                maybe_ag(i - 1)
        back(BPC - 1)
        maybe_ag(BPC - 1)



def build_program(cfg, T, zero_bias=False, tbp=None):
    nc = bacc.Bacc(num_swdge_queues=4)
    HID, NW, NP, NT, BPC = cfg.HID, cfg.NW, cfg.NP, cfg.NT, cfg.BPC

    xT_e = nc.declare_dram_parameter("xT", [HID, NP], BF16, isOutput=False)
    w1_e = nc.declare_dram_parameter("w1e", [HID, NW], BF16, isOutput=False)
    w2_e = nc.declare_dram_parameter("w2e", [HID, NW], BF16, isOutput=False)
    b1_e = nc.declare_dram_parameter("b1b", [P, HID], F32, isOutput=False)
    b2_e = nc.declare_dram_parameter("b2b", [P, HID], F32, isOutput=False)
    isrc_e = nc.declare_dram_parameter("isrc", [BPC, P, 8 * T], I16, isOutput=False)
    isrc2_e = nc.declare_dram_parameter("isrc2", [BPC, P, 8 * T], I16, isOutput=False)
    iblk_e = nc.declare_dram_parameter("iblk", [BPC, P, 8], I16, isOutput=False)
    iblk2_e = nc.declare_dram_parameter("iblk2", [BPC, P, 8], I16, isOutput=False)
    loc_e = nc.declare_dram_parameter("loce", [BPC, P, T], I16, isOutput=False)
    se_e = nc.declare_dram_parameter("sede", [BPC, P, 1], F32, isOutput=False)
    out_e = nc.declare_dram_parameter("out", [BPC * P, HID], F32, isOutput=True)

    tabf1 = nc.dram_tensor("tabf1", [NP, cfg.FAT], BF16)
    fat2own = nc.dram_tensor("fat2own", [BPC * P, cfg.FAT], BF16)
    tabf2sh = nc.dram_tensor("tabf2sh", [NP, cfg.FAT], BF16,
                             addr_space="Shared")

    with tile.TileContext(nc) as tc:
        with tc.tile_pool(name="const", bufs=1) as cp:
            cn = {}
            for nm, src in (("w1a", w1_e), ("w1b", w1_e), ("w2a", w2_e),
                            ("w2b", w2_e)):
                t = cp.tile([P, NW], BF16, tag=nm)
                lo = 0 if nm.endswith("a") else P
                nc.sync.dma_start(t[:], src[lo : lo + P, :])
                cn[nm] = t
            b1t = cp.tile([P, HID], F32)
            nc.sync.dma_start(b1t[:], b1_e[:, :])
            cn["b1t"] = b1t
            b2t = cp.tile([P, HID], F32)
            nc.sync.dma_start(b2t[:], b2_e[:, :])
            cn["b2t"] = b2t
            idn = cp.tile([P, P], BF16)
            make_identity(nc, idn[:])
            cn["idn"] = idn
            # I minus subdiagonal: D = sdif.T @ v telescopes run-starts
            sdif = cp.tile([P, P], BF16)
            make_identity(nc, sdif[:])
            nc.gpsimd.affine_select(
                out=sdif[:], in_=sdif[:], compare_op=OP.not_equal,
                fill=-1.0, base=1, pattern=[[-1, P]], channel_multiplier=1,
            )
            cn["sdif"] = sdif
            iota_dt = cp.tile([P, P, T], I16)
            nc.gpsimd.iota(iota_dt[:], pattern=[[1, P], [0, T]],
                           channel_multiplier=0)
            cn["iota_dt"] = iota_dt
            iota_e = cp.tile([P, T * P], I16)
            nc.gpsimd.iota(iota_e[:], pattern=[[1, T * P]],
                           channel_multiplier=0)
            cn["iota_e"] = iota_e

            # ---- layer-1 tables (full, redundant per core) ----
            PAN = 10
            with (
                tc.tile_pool(name="s1", bufs=3) as sp,
                tc.tile_pool(name="fp1", bufs=6) as fp,
                tc.tile_pool(name="ps1", bufs=6, space="PSUM") as pp,
            ):
                for pan in range(-(-NT // PAN)):
                    j0, j1 = pan * PAN, min(NT, (pan + 1) * PAN)
                    w = (j1 - j0) * P
                    xp0 = sp.tile([P, PAN * P], BF16, tag="xp0")
                    nc.sync.dma_start(xp0[:, :w], xT_e[0:P, j0 * P : j1 * P])
                    xp1 = sp.tile([P, PAN * P], BF16, tag="xp1")
                    nc.sync.dma_start(xp1[:, :w], xT_e[P : 2 * P, j0 * P : j1 * P])
                    for j in range(j0, j1):
                        o = (j - j0) * P
                        ps = pp.tile([P, NW], F32, tag="tps")
                        nc.tensor.matmul(ps[:], xp0[:, o : o + P], cn["w1a"][:],
                                         start=True, stop=False)
                        nc.tensor.matmul(ps[:], xp1[:, o : o + P], cn["w1b"][:],
                                         start=False, stop=True)
                        fat = fp.tile([P, cfg.FAT], BF16, tag="fat")
                        if j % 2 == 0:
                            nc.vector.tensor_copy(fat[:, 0:NW], ps[:])
                            nc.sync.dma_start(
                                tabf1[j * P : (j + 1) * P, :], fat[:])
                        else:
                            nc.scalar.activation(fat[:, 0:NW], ps[:], AF.Copy)
                            nc.scalar.dma_start(
                                tabf1[j * P : (j + 1) * P, :], fat[:])

            cn["zero_bias"] = zero_bias
            cn["tbp"] = tbp or [T] * cfg.BPC
            _edge_phase(nc, tc, cfg, T, cn, 1, tabf1, isrc_e, iblk_e, loc_e,
                        se_e, fat2own=fat2own, tabf2sh=tabf2sh)
            _edge_phase(nc, tc, cfg, T, cn, 2, tabf2sh, isrc2_e, iblk2_e,
                        loc_e, se_e, out_e=out_e)
    nc.finalize()
    return nc


# --------------------------------------------------------------------------
# Entry point
# --------------------------------------------------------------------------
def run_gat(inputs, cfg=None, trace=False):
    cfg = cfg or Cfg()
    in_maps, T, zero_bias, tbp = preprocess(cfg, **inputs)
    nc = build_program(cfg, T, zero_bias, tbp)
    res = run_bass_kernel_spmd(nc, in_maps, list(range(cfg.NC)), trace=trace)
    out = np.concatenate([res.results[c]["out"] for c in range(cfg.NC)], axis=0)
    return out[: cfg.N], res


def kernel(**inputs) -> np.ndarray:
    out, _ = run_gat(inputs)
    return np.ascontiguousarray(out, dtype=np.float32)
